# revision 10
# baseline (speedup 1.0000x reference)
"""Trainium2 Bass kernel for dual cross-attention + mean-fuse MLP (CAFM).

Problem: B=16, C=256, H*W=N=2048, DIM=256.
  out_1 = cross_attn(stft_seq, cqt_seq, wq1, wq2, wq3)   # [B, N, C]
  out_2 = cross_attn(cqt_seq, stft_seq, wq4, wq5, wq6)
  fused = concat([mean_n(out_1), mean_n(out_2)])         # [B, 512]
  out   = relu(fused @ W1 + b1) @ W2 + b2                # [B, 256]

Key algebra (exact):
  * softmax is invariant to per-row constants, so
      S = (X Wq + bq)(Y Wk + bk)^T * s  ~  (X A + 1 w^T) Y^T
    with A = s Wq Wk^T, w = s Wk bq — the K projection disappears.
  * only mean_n(softmax(S) V) is needed:
      y = p^T V,  p[m] = sum_n exp(S[n,m]) / rowsum[n]
    (the 1/N and the bv term are folded into the out1 linear on host:
     w1' = w1/N, b1' = b1 + concat(bv1,bv2) @ w1).

v2: the exp work is split across three engines per 128-row block:
  * cols 0-1024: ScalarE exact exp -> fp16, accum_out gives the rowsum.
  * cols 1024-1536 (DVE) and 1536-2048 (GpSimd): Schraudolph-style
    magic exp: i16 = round(S*1477.32 + 15293) bit-viewed as fp16 is
    2^(S/ln2) with a piecewise-linear mantissa (~1.8% rms, unbiased by
    constant choice; errors cancel in the softmax ratio and average out
    across 2048 rows — end-to-end ~1e-4).  One fused tensor_scalar op
    each, int16 output = packed fp16.  DVE reduces the packed fp16 for
    the magic half's rowsum at 2x rate.
  * p accumulation: 4 concurrent M=1 matmuls (col-tiled at partitions
    0/32/64/96 of one PSUM bank), lhsT = 1/rowsum in fp16.
  * all big matmuls (T~, scores, V) in fp8 with DoubleRow (K=256/pass).

Sharding: data-parallel over batch, 2 batch elements per core, both
attention directions per core. No collectives.

PSUM: 4 banks scores-ACT (2 bufs) + 2 banks scores-magic + 1 bank pacc
+ 1 bank T~/V/transpose scratch = 8.
"""

import numpy as np
import ml_dtypes

import concourse.bass as bass
import concourse.mybir as mybir
import concourse.tile as tile
from concourse.bass_utils import run_bass_kernel_spmd

F32 = mybir.dt.float32
F32R = mybir.dt.float32r
F16 = mybir.dt.float16
I16 = mybir.dt.int16
FP8 = mybir.dt.float8e4
DR = mybir.MatmulPerfMode.DoubleRow
AF = mybir.ActivationFunctionType
ALU = mybir.AluOpType

N = 2048          # sequence length (H*W)
C = 256           # channels
BLOCKS = N // 128  # 16 row blocks

KMAG = float(1024.0 / np.log(2.0))
CMAG = float(15360.0 - 1024.0 * 0.0437)


def split_multi_waits(nc):
    """This container's walrus accepts at most 1 sync-wait per instruction
    (2 for EventSemaphore). Tile's tail drain can carry more; move the
    excess onto preceding wait-only NoOps on the same engine."""
    f = nc.m.functions[0]
    n_new = 0
    for bb in f.blocks:
        insts = bb.instructions
        new_list = []
        changed = False
        for inst in insts:
            si = inst.sync_info
            waits = list(si.on_wait) if si and si.on_wait else []
            cap = 2 if isinstance(inst, mybir.InstEventSemaphore) else 1
            if len(waits) > cap:
                for w in waits[:-cap]:
                    nop = mybir.InstNoOp(
                        name=f"I-sw{n_new}-{inst.name}", ins=[], outs=[])
                    n_new += 1
                    nop.engine = inst.engine
                    nop.sync_info = mybir.SyncInfo(on_wait=[w], on_update=[])
                    new_list.append(nop)
                si.on_wait = waits[-cap:]
                inst.sync_info = si
                changed = True
            new_list.append(inst)
        if changed:
            bb.instructions = new_list
    return n_new


def build_nc(reps=1, for_sim=False):
    nc = bass.Bass("TRN2", target_bir_lowering=False, debug=False)

    # --- DRAM I/O (per core) --- all fp8 operands are host-prepped.
    xq8_d = nc.dram_tensor("xq8", [2, C, N], FP8, kind="ExternalInput")
    xk8_d = nc.dram_tensor("xk8", [2, C, N], FP8, kind="ExternalInput")
    a8_d = [nc.dram_tensor(f"a8{d}", [128, 2, C], FP8, kind="ExternalInput")
            for d in range(2)]
    wt_d = [nc.dram_tensor(f"wt{d}", [C], F32, kind="ExternalInput")
            for d in range(2)]
    wv8_d = [nc.dram_tensor(f"wv8{d}", [128, 2, C], FP8, kind="ExternalInput")
             for d in range(2)]
    w1_d = nc.dram_tensor("w1", [2 * C, C], F32, kind="ExternalInput")
    b1_d = nc.dram_tensor("b1", [C], F32, kind="ExternalInput")
    w2_d = nc.dram_tensor("w2", [C, C], F32, kind="ExternalInput")
    b2_d = nc.dram_tensor("b2", [C], F32, kind="ExternalInput")
    out_d = nc.dram_tensor("out", [C, 2], F32, kind="ExternalOutput")

    with tile.TileContext(nc) as tc, nc.allow_low_precision(reason="fp8/f16"):
        with (
            tc.tile_pool(name="const", bufs=1) as const,
            tc.tile_pool(name="seq", bufs=1) as seqp,
            tc.tile_pool(name="tt", bufs=2) as ttp,
            tc.tile_pool(name="vv", bufs=2) as vvp,
            tc.tile_pool(name="e16", bufs=3) as e16p,
            tc.tile_pool(name="ti", bufs=3) as tip,
            tc.tile_pool(name="small", bufs=3) as smallp,
            tc.tile_pool(name="psA", bufs=2, space="PSUM") as psap,
            tc.tile_pool(name="psB", bufs=2, space="PSUM") as pbp,
            tc.tile_pool(name="pacc", bufs=1, space="PSUM") as paccp,
            tc.tile_pool(name="tv", bufs=1, space="PSUM") as tvp,
        ):
            # --- input loads (two DMA queues) ---
            a8_sb, wt_sb, wv8_sb = [], [], []
            for d in range(2):
                a8 = const.tile([128, 2, C], FP8, tag=f"a8{d}")
                (nc.sync if d == 0 else nc.scalar).dma_start(
                    out=a8, in_=a8_d[d].ap())
                a8_sb.append(a8)
                wt = const.tile([128, 2], F32, tag=f"wt{d}")
                (nc.sync if d == 0 else nc.scalar).dma_start(
                    out=wt, in_=wt_d[d].ap().rearrange("(t p) -> p t", p=128))
                wt_sb.append(wt)
                wv8 = const.tile([128, 2, C], FP8, tag=f"wv8{d}")
                nc.scalar.dma_start(out=wv8, in_=wv8_d[d].ap())
                wv8_sb.append(wv8)

            xq8s, xk8s = [], []
            for b in range(2):
                q8 = seqp.tile([128, 2, N], FP8, tag=f"xq8{b}",
                               name=f"xq8_{b}")
                (nc.sync if b == 0 else nc.scalar).dma_start(
                    out=q8,
                    in_=xq8_d.ap()[b].rearrange("(k p) n -> p k n", p=128))
                xq8s.append(q8)
                k8 = seqp.tile([128, 2, N], FP8, tag=f"xk8{b}",
                               name=f"xk8_{b}")
                (nc.sync if b == 0 else nc.scalar).dma_start(
                    out=k8,
                    in_=xk8_d.ap()[b].rearrange("(k p) n -> p k n", p=128))
                xk8s.append(k8)

            w1_sb = const.tile([128, 4, C], F32)
            nc.sync.dma_start(
                out=w1_sb, in_=w1_d.ap().rearrange("(k p) c -> p k c", p=128))
            b1_sb = const.tile([128, 2], F32)
            nc.sync.dma_start(
                out=b1_sb, in_=b1_d.ap().rearrange("(t p) -> p t", p=128))
            w2_sb = const.tile([128, 2, C], F32)
            nc.scalar.dma_start(
                out=w2_sb, in_=w2_d.ap().rearrange("(k p) c -> p k c", p=128))
            b2_sb = const.tile([128, 2], F32)
            nc.scalar.dma_start(
                out=b2_sb, in_=b2_d.ap().rearrange("(t p) -> p t", p=128))

            one_sb = const.tile([128, 1], F32)
            nc.vector.memset(one_sb, 1.0)
            ft_sb = const.tile([128, 8], F32)  # fused^T columns (k-chunk, b)

            insts = [(b, d) for _ in range(reps)
                     for b in range(2) for d in range(2)]
            tts = {}

            def emit_tt_chunk(gi, ct, j4, on_act):
                b, d = insts[gi]
                q8 = xq8s[b] if d == 0 else xk8s[b]
                if gi not in tts:
                    tts[gi] = ttp.tile([128, 2, N], FP8, tag="tt",
                                       name=f"tt{gi}")
                tt = tts[gi]
                ps = tvp.tile([128, 512], F32, tag="tv",
                              name=f"ttps{gi}_{ct}{j4}")
                nc.tensor.matmul(
                    ps, a8_sb[d][:, :, ct * 128:(ct + 1) * 128],
                    q8[:, :, j4 * 512:(j4 + 1) * 512],
                    start=True, stop=True, perf_mode=DR)
                dst = tt[:, ct, j4 * 512:(j4 + 1) * 512]
                if on_act:
                    nc.scalar.activation(
                        dst, ps, AF.Identity,
                        bias=wt_sb[d][:, ct:ct + 1], scale=16.0)
                else:
                    nc.vector.tensor_scalar(
                        dst, ps, 16.0, wt_sb[d][:, ct:ct + 1],
                        op0=ALU.mult, op1=ALU.add)

            # T~ for the first instance upfront
            for ct in range(2):
                for j4 in range(4):
                    emit_tt_chunk(0, ct, j4, on_act=True)

            for gi, (b, d) in enumerate(insts):
                k8 = xk8s[b] if d == 0 else xq8s[b]
                tt = tts.pop(gi)
                wv8 = wv8_sb[d]

                v = vvp.tile([128, BLOCKS, C], F32R, tag="v",
                             name=f"v{gi}")

                def emit_v_pair(pv):
                    ps = tvp.tile([128, 512], F32, tag="tv",
                                  name=f"vps{gi}_{pv}")
                    for h in range(2):
                        nc.tensor.matmul(
                            ps[:, h * C:(h + 1) * C],
                            k8[:, :, (2 * pv + h) * 128:(2 * pv + h + 1) * 128],
                            wv8, start=True, stop=True, perf_mode=DR)
                    dst = v[:, 2 * pv:2 * pv + 2, :]
                    if pv % 2 == 0:
                        nc.scalar.activation(dst, ps, AF.Identity)
                    else:
                        nc.vector.tensor_copy(dst, ps)

                bank = paccp.tile([128, 512], F32, tag="bank",
                                  name=f"bank{gi}")
                pending = None
                evac_act = (gi % 2 == 0)

                def do_pacc(p):
                    e16_, ti16_, rsum_, pnb = p
                    rinv_ = smallp.tile([128, 1], F16, tag="rinv",
                                        name=f"rinv{gi}_{pnb}")
                    nc.vector.reciprocal(rinv_, rsum_)
                    chunks = (e16_[:, 0:512], e16_[:, 512:1024],
                              ti16_[:, 0:512], ti16_[:, 512:1024])
                    for g, ch in enumerate(chunks):
                        nc.tensor.matmul(
                            bank[32 * g:32 * g + 1, :], rinv_, ch,
                            start=(pnb == 0), stop=(pnb == BLOCKS - 1),
                            tile_position=(0, 32 * g), skip_group_check=True)

                for nb in range(BLOCKS):
                    psA = psap.tile([128, 1024], F32, tag="psA",
                                    name=f"psA{gi}_{nb}")
                    for jj in range(2):
                        nc.tensor.matmul(
                            psA[:, jj * 512:(jj + 1) * 512],
                            tt[:, :, nb * 128:(nb + 1) * 128],
                            k8[:, :, jj * 512:(jj + 1) * 512],
                            start=True, stop=True, perf_mode=DR)
                    pBs = []
                    for jj in range(2):
                        pB = pbp.tile([128, 512], F32, tag="pB",
                                      name=f"pB{gi}_{nb}_{jj}")
                        nc.tensor.matmul(
                            pB, tt[:, :, nb * 128:(nb + 1) * 128],
                            k8[:, :, (2 + jj) * 512:(3 + jj) * 512],
                            start=True, stop=True, perf_mode=DR)
                        pBs.append(pB)

                    e16 = e16p.tile([128, 1024], F16, tag="e16",
                                    name=f"e16_{gi}_{nb}")
                    racc = smallp.tile([128, 1], F32, tag="racc")
                    nc.scalar.activation(e16, psA, AF.Exp, accum_out=racc)

                    ti = tip.tile([128, 1024], I16, tag="ti",
                                  name=f"ti{gi}_{nb}")
                    for jj in range(2):
                        nc.vector.tensor_scalar(
                            ti[:, jj * 512:(jj + 1) * 512], pBs[jj],
                            KMAG, CMAG, op0=ALU.mult, op1=ALU.add)
                    ti16 = ti.bitcast(F16)
                    zmag = smallp.tile([128, 1], F32, tag="zmag")
                    zscr = smallp.tile([128, 1024], F16, tag="zscr", bufs=1,
                                       name=f"zscr{gi}_{nb}")
                    nc.vector.tensor_scalar(
                        zscr, ti16, 1.0, 0.0, op0=ALU.mult, op1=ALU.add,
                        accum_out=zmag)
                    rsum = smallp.tile([128, 1], F32, tag="rsum")
                    nc.gpsimd.tensor_add(rsum, racc, zmag)

                    if nb < 8:
                        emit_v_pair(nb)
                    elif gi + 1 < len(insts):
                        emit_tt_chunk(gi + 1, (nb - 8) // 4, (nb - 8) % 4,
                                      on_act=True)

                    # one block behind: reciprocal + p-accumulate, so the
                    # DVE/PE queues never head-block on this block's racc
                    if pending is not None:
                        do_pacc(pending)
                    pending = (e16, ti16, rsum, nb)
                do_pacc(pending)

                # p -> sbuf row [1, 2048] (GpSimd copies, PSUM->SBUF)
                p_sb = smallp.tile([1, N], F32, tag="p", bufs=2,
                                   name=f"p{gi}")
                for g in range(4):
                    nc.scalar.activation(
                        p_sb[0:1, g * 512:(g + 1) * 512],
                        bank[32 * g:32 * g + 1, :], AF.Identity)

                # transpose p via k=1 matmuls, then y = p^T V
                ptp = tvp.tile([128, 512], F32, tag="tv", name=f"ptp{gi}")
                for j in range(BLOCKS):
                    nc.tensor.matmul(
                        ptp[:, j:j + 1], p_sb[0:1, j * 128:(j + 1) * 128],
                        one_sb[0:1, :], start=(j == 0),
                        stop=(j == BLOCKS - 1), skip_group_check=True)
                pt_sb = smallp.tile([128, 16], F32R, tag="pt")
                nc.vector.tensor_copy(pt_sb, ptp[:, :16])
                yps = tvp.tile([128, 512], F32, tag="tv", name=f"yps{gi}")
                for j in range(BLOCKS):
                    nc.tensor.matmul(
                        yps[0:1, :C], pt_sb[:, j:j + 1], v[:, j, :],
                        start=(j == 0), stop=(j == BLOCKS - 1),
                        skip_group_check=True)
                y_sb = smallp.tile([1, C], F32, tag="y")
                nc.vector.tensor_copy(y_sb, yps[0:1, :C])

                # fused^T columns via k=1 transpose matmuls
                for h in range(2):
                    fcol = tvp.tile([128, 512], F32, tag="tv",
                                    name=f"fcol{gi}{h}")
                    nc.tensor.matmul(
                        fcol[:, 0:1], y_sb[0:1, h * 128:(h + 1) * 128],
                        one_sb[0:1, :], start=True, stop=True,
                        skip_group_check=True)
                    k = 2 * d + h
                    nc.vector.tensor_copy(
                        ft_sb[:, 2 * k + b:2 * k + b + 1], fcol[:, 0:1])

            # --- final MLP on the two local batch rows ---
            h_sb = smallp.tile([128, 2, 2], F32, tag="h")
            for t in range(2):
                hps = tvp.tile([128, 512], F32, tag="tv", name=f"hps{t}")
                for k in range(4):
                    nc.tensor.matmul(
                        hps[:, 0:2], w1_sb[:, k, t * 128:(t + 1) * 128],
                        ft_sb[:, 2 * k:2 * k + 2],
                        start=(k == 0), stop=(k == 3), skip_group_check=True)
                nc.scalar.activation(
                    h_sb[:, t, :], hps[:, 0:2], AF.Relu,
                    bias=b1_sb[:, t:t + 1], scale=1.0)
            o_sb = smallp.tile([128, 2, 2], F32, tag="o")
            for t in range(2):
                ops = tvp.tile([128, 512], F32, tag="tv", name=f"ops{t}")
                for k in range(2):
                    nc.tensor.matmul(
                        ops[:, 0:2], w2_sb[:, k, t * 128:(t + 1) * 128],
                        h_sb[:, k, :],
                        start=(k == 0), stop=(k == 1), skip_group_check=True)
                nc.scalar.activation(
                    o_sb[:, t, :], ops[:, 0:2], AF.Identity,
                    bias=b2_sb[:, t:t + 1], scale=1.0)
            nc.sync.dma_start(
                out=out_d.ap().rearrange("(t p) b -> p t b", p=128), in_=o_sb)

    if not for_sim:
        split_multi_waits(nc)
    return nc


_NC = None


def _get_nc():
    global _NC
    if _NC is None:
        _NC = build_nc()
    return _NC


def prep_inputs(stft_feat, cqt_feat, wq1_w, wq1_b, wq2_w, wq2_b, wq3_w, wq3_b,
                wq4_w, wq4_b, wq5_w, wq5_b, wq6_w, wq6_b,
                out1_w, out1_b, out2_w, out2_b):
    B = stft_feat.shape[0]
    s = 1.0 / np.sqrt(np.float32(C))
    f32 = np.float32
    fp8 = ml_dtypes.float8_e4m3
    sigma = np.float32(16.0)  # fp8 range balancing; kv-side scaled by 1/16

    def dr_layout(m):  # [256, C] -> [128, 2, C] (ki low, ko high)
        return np.ascontiguousarray(
            m.reshape(2, 128, -1).transpose(1, 0, 2).astype(fp8))

    A1 = np.asarray(wq1_w @ wq2_w.T, f32) * s * sigma
    wt1 = np.asarray(wq2_w @ wq1_b, f32) * s * sigma
    A2 = np.asarray(wq4_w @ wq5_w.T, f32) * s * sigma
    wt2 = np.asarray(wq5_w @ wq4_b, f32) * s * sigma
    WV1 = np.asarray(wq3_w, f32) * sigma
    WV2 = np.asarray(wq6_w, f32) * sigma
    bv_cat = np.concatenate([np.asarray(wq3_b, f32), np.asarray(wq6_b, f32)])
    w1p = (np.asarray(out1_w, f32) / f32(N))
    b1p = np.asarray(out1_b, f32) + bv_cat @ np.asarray(out1_w, f32)

    common = dict(
        a80=dr_layout(A1), a81=dr_layout(A2),
        wt0=np.ascontiguousarray(wt1), wt1=np.ascontiguousarray(wt2),
        wv80=dr_layout(WV1), wv81=dr_layout(WV2),
        w1=np.ascontiguousarray(w1p),
        b1=np.ascontiguousarray(b1p),
        w2=np.ascontiguousarray(np.asarray(out2_w, f32)),
        b2=np.ascontiguousarray(np.asarray(out2_b, f32)),
    )
    stft8 = (np.asarray(stft_feat, f32).reshape(B, C, N) / sigma).astype(fp8)
    cqt8 = (np.asarray(cqt_feat, f32).reshape(B, C, N) / sigma).astype(fp8)
    in_maps = []
    for i in range(8):
        m = dict(common)
        m["xq8"] = np.ascontiguousarray(stft8[2 * i:2 * i + 2])
        m["xk8"] = np.ascontiguousarray(cqt8[2 * i:2 * i + 2])
        in_maps.append(m)
    return in_maps


def kernel(**inputs):
    inputs = {k: np.asarray(v) for k, v in inputs.items()}
    B = inputs["stft_feat"].shape[0]
    nc = _get_nc()
    in_maps = prep_inputs(**inputs)
    res = run_bass_kernel_spmd(nc, in_maps, list(range(8)))
    out = np.empty((B, C), np.float32)
    for i in range(8):
        o = res.results[i]["out"]  # [C, 2]
        out[2 * i] = o[:, 0]
        out[2 * i + 1] = o[:, 1]
    return out


# revision 11
# speedup vs baseline: 1.2128x; 1.2128x over previous
"""Trainium2 Bass kernel for dual cross-attention + mean-fuse MLP (CAFM).

Problem: B=16, C=256, H*W=N=2048, DIM=256.
  out_1 = cross_attn(stft_seq, cqt_seq, wq1, wq2, wq3)   # [B, N, C]
  out_2 = cross_attn(cqt_seq, stft_seq, wq4, wq5, wq6)
  fused = concat([mean_n(out_1), mean_n(out_2)])         # [B, 512]
  out   = relu(fused @ W1 + b1) @ W2 + b2                # [B, 256]

Key algebra (exact):
  * softmax is invariant to per-row constants, so
      S = (X Wq + bq)(Y Wk + bk)^T * s  ~  (X A + 1 w^T) Y^T
    with A = s Wq Wk^T, w = s Wk bq — the K projection disappears.
  * only mean_n(softmax(S) V) is needed:
      y = p^T V,  p[m] = sum_n exp(S[n,m]) / rowsum[n]
    (the 1/N and the bv term are folded into the out1 linear on host:
     w1' = w1/N, b1' = b1 + concat(bv1,bv2) @ w1).

v2: the exp work is split across three engines per 128-row block:
  * cols 0-1024: ScalarE exact exp -> fp16, accum_out gives the rowsum.
  * cols 1024-1536 (DVE) and 1536-2048 (GpSimd): Schraudolph-style
    magic exp: i16 = round(S*1477.32 + 15293) bit-viewed as fp16 is
    2^(S/ln2) with a piecewise-linear mantissa (~1.8% rms, unbiased by
    constant choice; errors cancel in the softmax ratio and average out
    across 2048 rows — end-to-end ~1e-4).  One fused tensor_scalar op
    each, int16 output = packed fp16.  DVE reduces the packed fp16 for
    the magic half's rowsum at 2x rate.
  * p accumulation: 4 concurrent M=1 matmuls (col-tiled at partitions
    0/32/64/96 of one PSUM bank), lhsT = 1/rowsum in fp16.
  * all big matmuls (T~, scores, V) in fp8 with DoubleRow (K=256/pass).

Sharding: data-parallel over batch, 2 batch elements per core, both
attention directions per core. No collectives.

PSUM: 4 banks scores-ACT (2 bufs) + 2 banks scores-magic + 1 bank pacc
+ 1 bank T~/V/transpose scratch = 8.
"""

import numpy as np
import ml_dtypes

import concourse.bass as bass
import concourse.mybir as mybir
import concourse.tile as tile
from concourse.bass_utils import run_bass_kernel_spmd

F32 = mybir.dt.float32
F32R = mybir.dt.float32r
F16 = mybir.dt.float16
I16 = mybir.dt.int16
FP8 = mybir.dt.float8e4
DR = mybir.MatmulPerfMode.DoubleRow
AF = mybir.ActivationFunctionType
ALU = mybir.AluOpType

N = 2048          # sequence length (H*W)
C = 256           # channels
BLOCKS = N // 128  # 16 row blocks

KMAG = float(1024.0 / np.log(2.0))
CMAG = float(15360.0 - 1024.0 * 0.0437)


def split_multi_waits(nc):
    """This container's walrus accepts at most 1 sync-wait per instruction
    (2 for EventSemaphore). Tile's tail drain can carry more; move the
    excess onto preceding wait-only NoOps on the same engine."""
    f = nc.m.functions[0]
    n_new = 0
    for bb in f.blocks:
        insts = bb.instructions
        new_list = []
        changed = False
        for inst in insts:
            si = inst.sync_info
            waits = list(si.on_wait) if si and si.on_wait else []
            cap = 2 if isinstance(inst, mybir.InstEventSemaphore) else 1
            if len(waits) > cap:
                for w in waits[:-cap]:
                    nop = mybir.InstNoOp(
                        name=f"I-sw{n_new}-{inst.name}", ins=[], outs=[])
                    n_new += 1
                    nop.engine = inst.engine
                    nop.sync_info = mybir.SyncInfo(on_wait=[w], on_update=[])
                    new_list.append(nop)
                si.on_wait = waits[-cap:]
                inst.sync_info = si
                changed = True
            new_list.append(inst)
        if changed:
            bb.instructions = new_list
    return n_new


def build_nc(reps=1, for_sim=False):
    nc = bass.Bass("TRN2", target_bir_lowering=False, debug=False)

    # --- DRAM I/O (per core) --- all fp8 operands are host-prepped.
    xq8_d = nc.dram_tensor("xq8", [2, C, N], FP8, kind="ExternalInput")
    xk8_d = nc.dram_tensor("xk8", [2, C, N], FP8, kind="ExternalInput")
    a8_d = [nc.dram_tensor(f"a8{d}", [128, 2, C], FP8, kind="ExternalInput")
            for d in range(2)]
    wt_d = [nc.dram_tensor(f"wt{d}", [C], F32, kind="ExternalInput")
            for d in range(2)]
    wv8_d = [nc.dram_tensor(f"wv8{d}", [128, 2, C], FP8, kind="ExternalInput")
             for d in range(2)]
    w1_d = nc.dram_tensor("w1", [2 * C, C], F32, kind="ExternalInput")
    b1_d = nc.dram_tensor("b1", [C], F32, kind="ExternalInput")
    w2_d = nc.dram_tensor("w2", [C, C], F32, kind="ExternalInput")
    b2_d = nc.dram_tensor("b2", [C], F32, kind="ExternalInput")
    out_d = nc.dram_tensor("out", [C, 2], F32, kind="ExternalOutput")

    with tile.TileContext(nc) as tc, nc.allow_low_precision(reason="fp8/f16"):
        with (
            tc.tile_pool(name="const", bufs=1) as const,
            tc.tile_pool(name="seq", bufs=1) as seqp,
            tc.tile_pool(name="tt", bufs=2) as ttp,
            tc.tile_pool(name="vv", bufs=2) as vvp,
            tc.tile_pool(name="e16", bufs=3) as e16p,
            tc.tile_pool(name="ti", bufs=3) as tip,
            tc.tile_pool(name="small", bufs=3) as smallp,
            tc.tile_pool(name="psA", bufs=2, space="PSUM") as psap,
            tc.tile_pool(name="psB", bufs=2, space="PSUM") as pbp,
            tc.tile_pool(name="pacc", bufs=1, space="PSUM") as paccp,
            tc.tile_pool(name="tv", bufs=1, space="PSUM") as tvp,
        ):
            # --- input loads (two DMA queues) ---
            a8_sb, wt_sb, wv8_sb = [], [], []
            for d in range(2):
                a8 = const.tile([128, 2, C], FP8, tag=f"a8{d}")
                (nc.sync if d == 0 else nc.scalar).dma_start(
                    out=a8, in_=a8_d[d].ap())
                a8_sb.append(a8)
                wt = const.tile([128, 2], F32, tag=f"wt{d}")
                (nc.sync if d == 0 else nc.scalar).dma_start(
                    out=wt, in_=wt_d[d].ap().rearrange("(t p) -> p t", p=128))
                wt_sb.append(wt)
                wv8 = const.tile([128, 2, C], FP8, tag=f"wv8{d}")
                nc.scalar.dma_start(out=wv8, in_=wv8_d[d].ap())
                wv8_sb.append(wv8)

            xq8s, xk8s = [], []
            for b in range(2):
                q8 = seqp.tile([128, 2, N], FP8, tag=f"xq8{b}",
                               name=f"xq8_{b}")
                (nc.sync if b == 0 else nc.scalar).dma_start(
                    out=q8,
                    in_=xq8_d.ap()[b].rearrange("(k p) n -> p k n", p=128))
                xq8s.append(q8)
                k8 = seqp.tile([128, 2, N], FP8, tag=f"xk8{b}",
                               name=f"xk8_{b}")
                (nc.sync if b == 0 else nc.scalar).dma_start(
                    out=k8,
                    in_=xk8_d.ap()[b].rearrange("(k p) n -> p k n", p=128))
                xk8s.append(k8)

            w1_sb = const.tile([128, 4, C], F32)
            nc.sync.dma_start(
                out=w1_sb, in_=w1_d.ap().rearrange("(k p) c -> p k c", p=128))
            b1_sb = const.tile([128, 2], F32)
            nc.sync.dma_start(
                out=b1_sb, in_=b1_d.ap().rearrange("(t p) -> p t", p=128))
            w2_sb = const.tile([128, 2, C], F32)
            nc.scalar.dma_start(
                out=w2_sb, in_=w2_d.ap().rearrange("(k p) c -> p k c", p=128))
            b2_sb = const.tile([128, 2], F32)
            nc.scalar.dma_start(
                out=b2_sb, in_=b2_d.ap().rearrange("(t p) -> p t", p=128))

            one_sb = const.tile([128, 1], F32)
            nc.vector.memset(one_sb, 1.0)
            ft_sb = const.tile([128, 8], F32)  # fused^T columns (k-chunk, b)

            insts = [(b, d) for _ in range(reps)
                     for b in range(2) for d in range(2)]
            tts = {}
            deferred = []

            def emit_endgame(p_sb, v_, d_, gi_):
                ptp = tvp.tile([128, 512], F32, tag="tv", name=f"ptp{gi_}")
                for j in range(BLOCKS):
                    nc.tensor.matmul(
                        ptp[:, j:j + 1], p_sb[0:1, j * 128:(j + 1) * 128],
                        one_sb[0:1, :], start=(j == 0),
                        stop=(j == BLOCKS - 1), skip_group_check=True)
                pt_sb = smallp.tile([128, 16], F32R, tag="pt")
                nc.vector.tensor_copy(pt_sb, ptp[:, :16])
                yps = tvp.tile([128, 512], F32, tag="tv", name=f"yps{gi_}")
                for j in range(BLOCKS):
                    nc.tensor.matmul(
                        yps[0:1, :C], pt_sb[:, j:j + 1], v_[:, j, :],
                        start=(j == 0), stop=(j == BLOCKS - 1),
                        skip_group_check=True)
                y_sb = smallp.tile([1, C], F32, tag="y")
                nc.vector.tensor_copy(y_sb, yps[0:1, :C])
                for h in range(2):
                    fcol = tvp.tile([128, 512], F32, tag="tv",
                                    name=f"fcol{gi_}{h}")
                    nc.tensor.matmul(
                        fcol[:, 0:1], y_sb[0:1, h * 128:(h + 1) * 128],
                        one_sb[0:1, :], start=True, stop=True,
                        skip_group_check=True)
                    k = 2 * d_ + h
                    b_ = insts[gi_][0]
                    nc.vector.tensor_copy(
                        ft_sb[:, 2 * k + b_:2 * k + b_ + 1], fcol[:, 0:1])

            def emit_tt_chunk(gi, ct, j4, on_act):
                b, d = insts[gi]
                q8 = xq8s[b] if d == 0 else xk8s[b]
                if gi not in tts:
                    tts[gi] = ttp.tile([128, 2, N], FP8, tag="tt",
                                       name=f"tt{gi}")
                tt = tts[gi]
                ps = tvp.tile([128, 512], F32, tag="tv",
                              name=f"ttps{gi}_{ct}{j4}")
                nc.tensor.matmul(
                    ps, a8_sb[d][:, :, ct * 128:(ct + 1) * 128],
                    q8[:, :, j4 * 512:(j4 + 1) * 512],
                    start=True, stop=True, perf_mode=DR)
                dst = tt[:, ct, j4 * 512:(j4 + 1) * 512]
                if on_act:
                    nc.scalar.activation(
                        dst, ps, AF.Identity,
                        bias=wt_sb[d][:, ct:ct + 1], scale=16.0)
                else:
                    nc.vector.tensor_scalar(
                        dst, ps, 16.0, wt_sb[d][:, ct:ct + 1],
                        op0=ALU.mult, op1=ALU.add)

            # T~ for the first instance upfront
            for ct in range(2):
                for j4 in range(4):
                    emit_tt_chunk(0, ct, j4, on_act=True)

            for gi, (b, d) in enumerate(insts):
                k8 = xk8s[b] if d == 0 else xq8s[b]
                tt = tts.pop(gi)
                wv8 = wv8_sb[d]

                v = vvp.tile([128, BLOCKS, C], F32R, tag="v",
                             name=f"v{gi}")

                def emit_v_pair(pv):
                    ps = tvp.tile([128, 512], F32, tag="tv",
                                  name=f"vps{gi}_{pv}")
                    for h in range(2):
                        nc.tensor.matmul(
                            ps[:, h * C:(h + 1) * C],
                            k8[:, :, (2 * pv + h) * 128:(2 * pv + h + 1) * 128],
                            wv8, start=True, stop=True, perf_mode=DR)
                    dst = v[:, 2 * pv:2 * pv + 2, :]
                    if pv % 2 == 0:
                        nc.scalar.activation(dst, ps, AF.Identity)
                    else:
                        nc.vector.tensor_copy(dst, ps)

                bank = paccp.tile([128, 512], F32, tag="bank",
                                  name=f"bank{gi}")
                pending = None
                evac_act = (gi % 2 == 0)

                def do_pacc(p):
                    e16_, ti16_, rsum_, pnb = p
                    rinv_ = smallp.tile([128, 1], F16, tag="rinv",
                                        name=f"rinv{gi}_{pnb}")
                    nc.vector.reciprocal(rinv_, rsum_)
                    chunks = (e16_[:, 0:512], e16_[:, 512:1024],
                              ti16_[:, 0:512], ti16_[:, 512:1024])
                    for g, ch in enumerate(chunks):
                        nc.tensor.matmul(
                            bank[32 * g:32 * g + 1, :], rinv_, ch,
                            start=(pnb == 0), stop=(pnb == BLOCKS - 1),
                            tile_position=(0, 32 * g), skip_group_check=True)

                for nb in range(BLOCKS):
                    if nb == 2 and deferred:
                        emit_endgame(*deferred.pop(0))
                    psA = psap.tile([128, 1024], F32, tag="psA",
                                    name=f"psA{gi}_{nb}")
                    for jj in range(2):
                        nc.tensor.matmul(
                            psA[:, jj * 512:(jj + 1) * 512],
                            tt[:, :, nb * 128:(nb + 1) * 128],
                            k8[:, :, jj * 512:(jj + 1) * 512],
                            start=True, stop=True, perf_mode=DR)
                    pBs = []
                    for jj in range(2):
                        pB = pbp.tile([128, 512], F32, tag="pB",
                                      name=f"pB{gi}_{nb}_{jj}")
                        nc.tensor.matmul(
                            pB, tt[:, :, nb * 128:(nb + 1) * 128],
                            k8[:, :, (2 + jj) * 512:(3 + jj) * 512],
                            start=True, stop=True, perf_mode=DR)
                        pBs.append(pB)

                    e16 = e16p.tile([128, 1024], F16, tag="e16",
                                    name=f"e16_{gi}_{nb}")
                    racc = smallp.tile([128, 1], F32, tag="racc")
                    nc.scalar.activation(e16, psA, AF.Exp, accum_out=racc)

                    ti = tip.tile([128, 1024], I16, tag="ti",
                                  name=f"ti{gi}_{nb}")
                    for jj in range(2):
                        nc.vector.tensor_scalar(
                            ti[:, jj * 512:(jj + 1) * 512], pBs[jj],
                            KMAG, CMAG, op0=ALU.mult, op1=ALU.add)
                    ti16 = ti.bitcast(F16)
                    zmag = smallp.tile([128, 1], F32, tag="zmag")
                    zscr = smallp.tile([128, 1024], F16, tag="zscr", bufs=1,
                                       name=f"zscr{gi}_{nb}")
                    nc.vector.tensor_scalar(
                        zscr, ti16, 1.0, 0.0, op0=ALU.mult, op1=ALU.add,
                        accum_out=zmag)
                    rsum = smallp.tile([128, 1], F32, tag="rsum")
                    nc.gpsimd.tensor_add(rsum, racc, zmag)

                    if nb < 8:
                        emit_v_pair(nb)
                    elif gi + 1 < len(insts):
                        emit_tt_chunk(gi + 1, (nb - 8) // 4, (nb - 8) % 4,
                                      on_act=True)

                    # one block behind: reciprocal + p-accumulate, so the
                    # DVE/PE queues never head-block on this block's racc
                    if pending is not None:
                        do_pacc(pending)
                    pending = (e16, ti16, rsum, nb)
                do_pacc(pending)

                # p -> sbuf row [1, 2048] (GpSimd copies, PSUM->SBUF)
                p_sb = smallp.tile([1, N], F32, tag="p", bufs=2,
                                   name=f"p{gi}")
                for g in range(4):
                    nc.scalar.activation(
                        p_sb[0:1, g * 512:(g + 1) * 512],
                        bank[32 * g:32 * g + 1, :], AF.Identity)

                # defer transpose-p / y / fused-cols into the next
                # instance's loop so PE's FIFO isn't blocked here
                deferred.append((p_sb, v, d, gi))

            while deferred:
                emit_endgame(*deferred.pop(0))

            # --- final MLP on the two local batch rows ---
            h_sb = smallp.tile([128, 2, 2], F32, tag="h")
            for t in range(2):
                hps = tvp.tile([128, 512], F32, tag="tv", name=f"hps{t}")
                for k in range(4):
                    nc.tensor.matmul(
                        hps[:, 0:2], w1_sb[:, k, t * 128:(t + 1) * 128],
                        ft_sb[:, 2 * k:2 * k + 2],
                        start=(k == 0), stop=(k == 3), skip_group_check=True)
                nc.scalar.activation(
                    h_sb[:, t, :], hps[:, 0:2], AF.Relu,
                    bias=b1_sb[:, t:t + 1], scale=1.0)
            o_sb = smallp.tile([128, 2, 2], F32, tag="o")
            for t in range(2):
                ops = tvp.tile([128, 512], F32, tag="tv", name=f"ops{t}")
                for k in range(2):
                    nc.tensor.matmul(
                        ops[:, 0:2], w2_sb[:, k, t * 128:(t + 1) * 128],
                        h_sb[:, k, :],
                        start=(k == 0), stop=(k == 1), skip_group_check=True)
                nc.scalar.activation(
                    o_sb[:, t, :], ops[:, 0:2], AF.Identity,
                    bias=b2_sb[:, t:t + 1], scale=1.0)
            nc.sync.dma_start(
                out=out_d.ap().rearrange("(t p) b -> p t b", p=128), in_=o_sb)

    if not for_sim:
        split_multi_waits(nc)
    return nc


_NC = None


def _get_nc():
    global _NC
    if _NC is None:
        _NC = build_nc()
    return _NC


def prep_inputs(stft_feat, cqt_feat, wq1_w, wq1_b, wq2_w, wq2_b, wq3_w, wq3_b,
                wq4_w, wq4_b, wq5_w, wq5_b, wq6_w, wq6_b,
                out1_w, out1_b, out2_w, out2_b):
    B = stft_feat.shape[0]
    s = 1.0 / np.sqrt(np.float32(C))
    f32 = np.float32
    fp8 = ml_dtypes.float8_e4m3
    sigma = np.float32(16.0)  # fp8 range balancing; kv-side scaled by 1/16

    def dr_layout(m):  # [256, C] -> [128, 2, C] (ki low, ko high)
        return np.ascontiguousarray(
            m.reshape(2, 128, -1).transpose(1, 0, 2).astype(fp8))

    A1 = np.asarray(wq1_w @ wq2_w.T, f32) * s * sigma
    wt1 = np.asarray(wq2_w @ wq1_b, f32) * s * sigma
    A2 = np.asarray(wq4_w @ wq5_w.T, f32) * s * sigma
    wt2 = np.asarray(wq5_w @ wq4_b, f32) * s * sigma
    WV1 = np.asarray(wq3_w, f32) * sigma
    WV2 = np.asarray(wq6_w, f32) * sigma
    bv_cat = np.concatenate([np.asarray(wq3_b, f32), np.asarray(wq6_b, f32)])
    w1p = (np.asarray(out1_w, f32) / f32(N))
    b1p = np.asarray(out1_b, f32) + bv_cat @ np.asarray(out1_w, f32)

    common = dict(
        a80=dr_layout(A1), a81=dr_layout(A2),
        wt0=np.ascontiguousarray(wt1), wt1=np.ascontiguousarray(wt2),
        wv80=dr_layout(WV1), wv81=dr_layout(WV2),
        w1=np.ascontiguousarray(w1p),
        b1=np.ascontiguousarray(b1p),
        w2=np.ascontiguousarray(np.asarray(out2_w, f32)),
        b2=np.ascontiguousarray(np.asarray(out2_b, f32)),
    )
    stft8 = (np.asarray(stft_feat, f32).reshape(B, C, N) / sigma).astype(fp8)
    cqt8 = (np.asarray(cqt_feat, f32).reshape(B, C, N) / sigma).astype(fp8)
    in_maps = []
    for i in range(8):
        m = dict(common)
        m["xq8"] = np.ascontiguousarray(stft8[2 * i:2 * i + 2])
        m["xk8"] = np.ascontiguousarray(cqt8[2 * i:2 * i + 2])
        in_maps.append(m)
    return in_maps


def kernel(**inputs):
    inputs = {k: np.asarray(v) for k, v in inputs.items()}
    B = inputs["stft_feat"].shape[0]
    nc = _get_nc()
    in_maps = prep_inputs(**inputs)
    res = run_bass_kernel_spmd(nc, in_maps, list(range(8)))
    out = np.empty((B, C), np.float32)
    for i in range(8):
        o = res.results[i]["out"]  # [C, 2]
        out[2 * i] = o[:, 0]
        out[2 * i + 1] = o[:, 1]
    return out
